# revision 3
# baseline (speedup 1.0000x reference)
"""Complex-valued multihead attention (B=4, T=1024, C=512, H=8) on 8 Trainium2
NeuronCores.

Sharding: core c = (b = c//2, half = c%2) handles batch b and heads
half*4 .. half*4+3 (tensor-parallel over heads within a batch).  The out_proj
is computed as per-core partials over each core's 256 head-dims and summed on
the host (the reduce of the head-TP all-gather), where the bo bias is added.

On-device math uses float32r (full-rate fp32 PE path, ~1e-4 component error).
Complex matmuls are computed as single K=128 matmuls by stacking real/imag
parts along the contraction axis, e.g.
  Sr = [Qr;Qi]^T-stationary x [Kr;-Ki]-moving  (t x s tile in one matmul).
Linear-layer biases are folded in as K=1 fp16 matmuls accumulating into PSUM.
"""
import os

import numpy as np

B, T, C, H = 4, 1024, 512, 8
D = C // H  # 64
O = 256     # head-dims per core (4 heads x 64)
SCALE = D ** (-0.5)
N_CORES = 8
IC = C // 128  # 4 contraction chunks over c_in
TP = T // 128  # 8 t-partition chunks
SF = T // 512  # 2 free-dim chunks of 512

_prog_cache = {}
last_exec_ns = None
last_scope_times = None


def _build_program():
    import concourse.bacc as bacc
    import concourse.tile as tile
    from concourse import mybir

    F32 = mybir.dt.float32
    F32R = mybir.dt.float32r
    F16 = mybir.dt.float16
    AF = mybir.ActivationFunctionType
    ALU = mybir.AluOpType

    nc = bacc.Bacc("TRN2", target_bir_lowering=False, debug=False,
                   num_devices=N_CORES)

    # ---- DRAM I/O ----
    xt_r = nc.dram_tensor("xt_r", [C, T], F32R, kind="ExternalInput").ap()
    xt_i = nc.dram_tensor("xt_i", [C, T], F32R, kind="ExternalInput").ap()
    wstk = {}
    for name in ("wq_a", "wq_b", "wk1_a", "wk1_b", "wk2_a", "wk2_b"):
        wstk[name] = nc.dram_tensor(name, [4, IC, 128, 128], F32R,
                                    kind="ExternalInput").ap()
    for name in ("wv1_a", "wv1_b", "wv2_a", "wv2_b"):
        wstk[name] = nc.dram_tensor(name, [IC, 128, 512], F32R,
                                    kind="ExternalInput").ap()
    wo_r = nc.dram_tensor("wo_r", [4, 128, C], F32R, kind="ExternalInput").ap()
    wo_i = nc.dram_tensor("wo_i", [4, 128, C], F32R, kind="ExternalInput").ap()
    bias_q = nc.dram_tensor("bias_q", [4, 128], F16, kind="ExternalInput").ap()
    bias_k1 = nc.dram_tensor("bias_k1", [4, 128], F16, kind="ExternalInput").ap()
    bias_k2 = nc.dram_tensor("bias_k2", [4, 128], F16, kind="ExternalInput").ap()
    bias_va = nc.dram_tensor("bias_va", [1, 512], F16, kind="ExternalInput").ap()
    bias_vb = nc.dram_tensor("bias_vb", [1, 512], F16, kind="ExternalInput").ap()
    ones_in = nc.dram_tensor("ones_in", [1, 512], F16, kind="ExternalInput").ap()

    attw_r = nc.dram_tensor("attw_r", [4, T, T], F32, kind="ExternalOutput").ap()
    attw_i = nc.dram_tensor("attw_i", [4, T, T], F32, kind="ExternalOutput").ap()
    yp_r = nc.dram_tensor("yp_r", [T, C], F32, kind="ExternalOutput").ap()
    yp_i = nc.dram_tensor("yp_i", [T, C], F32, kind="ExternalOutput").ap()

    with tile.TileContext(nc) as tc:
        with (
            tc.tile_pool(name="xt", bufs=1) as xt_pool,
            tc.tile_pool(name="wt", bufs=16) as wt_pool,
            tc.tile_pool(name="wv", bufs=8) as wv_pool,
            tc.tile_pool(name="wo", bufs=1) as wo_pool,
            tc.tile_pool(name="qk", bufs=2) as qk_pool,
            tc.tile_pool(name="vs", bufs=1) as vs_pool,
            tc.tile_pool(name="st", bufs=16) as st_pool,
            tc.tile_pool(name="sdr", bufs=4) as sdr_pool,
            tc.tile_pool(name="av", bufs=1) as av_pool,
            tc.tile_pool(name="yp", bufs=2) as yp_pool,
            tc.tile_pool(name="cst", bufs=1) as cst_pool,
            tc.tile_pool(name="pp", bufs=4, space="PSUM") as pp_pool,
            tc.tile_pool(name="pav", bufs=2, space="PSUM") as pav_pool,
        ):
            # ---- constants / inputs ----
            ones_sb = cst_pool.tile([1, 512], F16, tag="ones")
            nc.gpsimd.dma_start(ones_sb[:], ones_in[:])
            bias_sb = {}
            for nm, ap in (("bq", bias_q), ("bk1", bias_k1), ("bk2", bias_k2)):
                for lh in range(4):
                    t = cst_pool.tile([1, 128], F16, tag=f"{nm}{lh}")
                    nc.gpsimd.dma_start(t[:], ap[lh:lh + 1, :])
                    bias_sb[(nm, lh)] = t
            bva_sb = cst_pool.tile([1, 512], F16, tag="bva")
            nc.gpsimd.dma_start(bva_sb[:], bias_va[:])
            bvb_sb = cst_pool.tile([1, 512], F16, tag="bvb")
            nc.gpsimd.dma_start(bvb_sb[:], bias_vb[:])

            xtr = []
            xti = []
            for ic in range(IC):
                t_r = xt_pool.tile([128, T], F32R, tag=f"xtr{ic}")
                nc.gpsimd.dma_start(t_r[:], xt_r[ic * 128:(ic + 1) * 128, :])
                xtr.append(t_r)
                t_i = xt_pool.tile([128, T], F32R, tag=f"xti{ic}")
                nc.gpsimd.dma_start(t_i[:], xt_i[ic * 128:(ic + 1) * 128, :])
                xti.append(t_i)

            wo_sb = {}
            for lh in range(4):
                t = wo_pool.tile([128, C], F32R, tag=f"wor{lh}")
                nc.gpsimd.dma_start(t[:], wo_r[lh])
                wo_sb[("r", lh)] = t
                t = wo_pool.tile([128, C], F32R, tag=f"woi{lh}")
                nc.gpsimd.dma_start(t[:], wo_i[lh])
                wo_sb[("i", lh)] = t

            # ---- vstk projections, all 4 heads batched in the free dim ----
            # per tp: vstk_a = [Vr_h|Vi_h]x4 heads, vstk_b = [-Vi_h|Vr_h]x4
            vstk = {}
            for variant, (wa, wb, bseg) in {
                "a": ("wv1_a", "wv1_b", bva_sb),
                "b": ("wv2_a", "wv2_b", bvb_sb),
            }.items():
                wa_t = []
                wb_t = []
                for ic in range(IC):
                    t = wv_pool.tile([128, 512], F32R, tag="wv")
                    nc.gpsimd.dma_start(t[:], wstk[wa][ic])
                    wa_t.append(t)
                    t = wv_pool.tile([128, 512], F32R, tag="wv")
                    nc.gpsimd.dma_start(t[:], wstk[wb][ic])
                    wb_t.append(t)
                for tp in range(TP):
                    ps = pp_pool.tile([128, 512], F32, tag="pp")
                    for ic in range(IC):
                        nc.tensor.matmul(
                            ps[:], xtr[ic][:, tp * 128:(tp + 1) * 128],
                            wa_t[ic][:], start=(ic == 0), stop=False)
                        nc.tensor.matmul(
                            ps[:], xti[ic][:, tp * 128:(tp + 1) * 128],
                            wb_t[ic][:], start=False, stop=False)
                    nc.tensor.matmul(
                        ps[:], ones_sb[:, 0:128], bseg[:],
                        start=False, stop=True)
                    dst = vs_pool.tile([128, 512], F32R,
                                       tag=f"v{variant}_{tp}")
                    nc.scalar.activation(dst[:], ps[:], AF.Identity,
                                         bias=0.0, scale=1.0)
                    vstk[(variant, tp)] = dst

            avstk = {}
            # ---- per-head: Q/K projections, scores, AV ----
            for lh in range(4):
                qkt = {}
                for name, (wa, wb, bnm) in {
                    "qt": ("wq_a", "wq_b", "bq"),
                    "ktn": ("wk1_a", "wk1_b", "bk1"),
                    "kts": ("wk2_a", "wk2_b", "bk2"),
                }.items():
                    wa_t = []
                    wb_t = []
                    for ic in range(IC):
                        t = wt_pool.tile([128, 128], F32R, tag="w")
                        nc.gpsimd.dma_start(t[:], wstk[wa][lh, ic])
                        wa_t.append(t)
                        t = wt_pool.tile([128, 128], F32R, tag="w")
                        nc.gpsimd.dma_start(t[:], wstk[wb][lh, ic])
                        wb_t.append(t)
                    dst = qk_pool.tile([128, T], F32R, tag=name)
                    for tf in range(SF):
                        ps = pp_pool.tile([128, 512], F32, tag="pp")
                        for ic in range(IC):
                            nc.tensor.matmul(
                                ps[:], wa_t[ic][:],
                                xtr[ic][:, tf * 512:(tf + 1) * 512],
                                start=(ic == 0), stop=False)
                            nc.tensor.matmul(
                                ps[:], wb_t[ic][:],
                                xti[ic][:, tf * 512:(tf + 1) * 512],
                                start=False, stop=False)
                        nc.tensor.matmul(
                            ps[:], bias_sb[(bnm, lh)][:], ones_sb[:],
                            start=False, stop=True)
                        nc.scalar.activation(dst[:, tf * 512:(tf + 1) * 512],
                                             ps[:], AF.Identity,
                                             bias=0.0, scale=1.0)
                    qkt[name] = dst
                qt, ktn, kts = qkt["qt"], qkt["ktn"], qkt["kts"]

                # St' = relu(SCALE * S^T) tiles [s-part x t] (feeds AV), per
                # t-half to bound SBUF.
                for th in range(2):
                    stt = {}
                    for sp in range(8):
                        ps_r = pp_pool.tile([128, 512], F32, tag="pp")
                        nc.tensor.matmul(
                            ps_r[:], ktn[:, sp * 128:(sp + 1) * 128],
                            qt[:, th * 512:(th + 1) * 512],
                            start=True, stop=True)
                        ps_i = pp_pool.tile([128, 512], F32, tag="pp")
                        nc.tensor.matmul(
                            ps_i[:], kts[:, sp * 128:(sp + 1) * 128],
                            qt[:, th * 512:(th + 1) * 512],
                            start=True, stop=True)
                        st_r = st_pool.tile([128, 512], F32R, tag="st")
                        nc.scalar.activation(st_r[:], ps_r[:], AF.Relu,
                                             bias=0.0, scale=SCALE)
                        st_i = st_pool.tile([128, 512], F32R, tag="st")
                        nc.scalar.activation(st_i[:], ps_i[:], AF.Relu,
                                             bias=0.0, scale=SCALE)
                        stt[("r", sp)] = st_r
                        stt[("i", sp)] = st_i

                    # S' = relu(SCALE * S) tiles [t-part x s] for the t-half;
                    # streamed straight to the attn_weights output.
                    for tpl in range(4):
                        tp = th * 4 + tpl
                        for sf in range(SF):
                            ps_r = pp_pool.tile([128, 512], F32, tag="pp")
                            nc.tensor.matmul(
                                ps_r[:], qt[:, tp * 128:(tp + 1) * 128],
                                ktn[:, sf * 512:(sf + 1) * 512],
                                start=True, stop=True)
                            ps_i = pp_pool.tile([128, 512], F32, tag="pp")
                            nc.tensor.matmul(
                                ps_i[:], qt[:, tp * 128:(tp + 1) * 128],
                                kts[:, sf * 512:(sf + 1) * 512],
                                start=True, stop=True)
                            s_r = sdr_pool.tile([128, 512], F32, tag="sdr")
                            nc.vector.tensor_scalar(s_r[:], ps_r[:], SCALE, 0.0,
                                                    ALU.mult, ALU.max)
                            s_i = sdr_pool.tile([128, 512], F32, tag="sdr")
                            nc.vector.tensor_scalar(s_i[:], ps_i[:], SCALE, 0.0,
                                                    ALU.mult, ALU.max)
                            nc.sync.dma_start(
                                attw_r[lh, tp * 128:(tp + 1) * 128,
                                       sf * 512:(sf + 1) * 512], s_r[:])
                            nc.gpsimd.dma_start(
                                attw_i[lh, tp * 128:(tp + 1) * 128,
                                       sf * 512:(sf + 1) * 512], s_i[:])

                    # AV for this t-half: avstk psum rows = [AVt_r; AVt_i]
                    ps_av = pav_pool.tile([128, 512], F32, tag="pav")
                    for sp in range(8):
                        nc.tensor.matmul(
                            ps_av[:],
                            vstk[("a", sp)][:, lh * 128:(lh + 1) * 128],
                            stt[("r", sp)][:],
                            start=(sp == 0), stop=False)
                        nc.tensor.matmul(
                            ps_av[:],
                            vstk[("b", sp)][:, lh * 128:(lh + 1) * 128],
                            stt[("i", sp)][:],
                            start=False, stop=(sp == 7))
                    if th == 0:
                        av_sb = av_pool.tile([128, T], F32R, tag=f"av{lh}")
                        avstk[lh] = av_sb
                    nc.scalar.activation(
                        avstk[lh][:, th * 512:(th + 1) * 512], ps_av[:],
                        AF.Identity, bias=0.0, scale=1.0)

            # ---- out_proj partials ----
            for tp in range(TP):
                ps_r = pp_pool.tile([128, 512], F32, tag="pp")
                ps_i = pp_pool.tile([128, 512], F32, tag="pp")
                for lh in range(4):
                    nc.tensor.matmul(ps_r[:],
                                     avstk[lh][:, tp * 128:(tp + 1) * 128],
                                     wo_sb[("r", lh)][:],
                                     start=(lh == 0), stop=(lh == 3))
                for lh in range(4):
                    nc.tensor.matmul(ps_i[:],
                                     avstk[lh][:, tp * 128:(tp + 1) * 128],
                                     wo_sb[("i", lh)][:],
                                     start=(lh == 0), stop=(lh == 3))
                o_r = yp_pool.tile([128, 512], F32, tag="yp")
                nc.vector.tensor_copy(o_r[:], ps_r[:])
                nc.sync.dma_start(yp_r[tp * 128:(tp + 1) * 128, :], o_r[:])
                o_i = yp_pool.tile([128, 512], F32, tag="yp")
                nc.vector.tensor_copy(o_i[:], ps_i[:])
                nc.gpsimd.dma_start(yp_i[tp * 128:(tp + 1) * 128, :], o_i[:])

    nc.compile()
    return nc


def _head_stacks(Wr, Wi, hs):
    # -> per-head (512, 128) transposed stationary blocks
    return Wr[hs, :].T.copy(), Wi[hs, :].T.copy()


def _as_chunks(m):
    # (512, 128) -> (IC, 128, 128)
    return np.ascontiguousarray(m.reshape(IC, 128, 128))


def _core_inputs(query, Wq, bq, Wk, bk, Wv, bv, Wo, bo, b, half):
    f32 = np.float32
    x = query[b]
    xt_r = np.ascontiguousarray(x.real.T).astype(f32)
    xt_i = np.ascontiguousarray(x.imag.T).astype(f32)

    WqT_r, WqT_i = Wq.real.T.astype(f32), Wq.imag.T.astype(f32)
    WkT_r, WkT_i = Wk.real.T.astype(f32), Wk.imag.T.astype(f32)
    WvT_r, WvT_i = Wv.real.T.astype(f32), Wv.imag.T.astype(f32)
    WoT_r, WoT_i = Wo.real.T.astype(f32), Wo.imag.T.astype(f32)

    def stacks(WT_r, WT_i, a_pair, b_pair):
        # a_pair/b_pair: ((sign, 'r'|'i'), (sign, 'i'|'r')) selecting the
        # [left|right] 64-col halves of the xr-term and xi-term stationaries
        src = {"r": WT_r, "i": WT_i}
        a_l = []
        b_l = []
        for lh in range(4):
            g = half * 4 + lh
            hs = slice(g * D, (g + 1) * D)
            cols_a = np.concatenate(
                [s * src[k][:, hs] for s, k in a_pair], axis=1)
            cols_b = np.concatenate(
                [s * src[k][:, hs] for s, k in b_pair], axis=1)
            a_l.append(_as_chunks(cols_a))
            b_l.append(_as_chunks(cols_b))
        return np.stack(a_l), np.stack(b_l)

    wq_a, wq_b = stacks(WqT_r, WqT_i, ((1, "r"), (1, "i")), ((-1, "i"), (1, "r")))
    wk1_a, wk1_b = stacks(WkT_r, WkT_i, ((1, "r"), (-1, "i")), ((-1, "i"), (-1, "r")))
    wk2_a, wk2_b = stacks(WkT_r, WkT_i, ((1, "i"), (1, "r")), ((1, "r"), (-1, "i")))

    def vstacks(a_pair, b_pair):
        src_ = {"r": WvT_r, "i": WvT_i}
        cols_a = []
        cols_b = []
        for lh in range(4):
            g = half * 4 + lh
            hs = slice(g * D, (g + 1) * D)
            cols_a.append(np.concatenate(
                [s * src_[k][:, hs] for s, k in a_pair], axis=1))
            cols_b.append(np.concatenate(
                [s * src_[k][:, hs] for s, k in b_pair], axis=1))
        a = np.ascontiguousarray(
            np.concatenate(cols_a, axis=1).reshape(IC, 128, 512))
        bm = np.ascontiguousarray(
            np.concatenate(cols_b, axis=1).reshape(IC, 128, 512))
        return a, bm

    wv1_a, wv1_b = vstacks(((1, "r"), (1, "i")), ((-1, "i"), (1, "r")))
    wv2_a, wv2_b = vstacks(((-1, "i"), (1, "r")), ((-1, "r"), (-1, "i")))

    wo_r_l = []
    wo_i_l = []
    bias = {}
    for nm in ("bias_q", "bias_k1", "bias_k2"):
        bias[nm] = np.zeros((4, 128), np.float16)
    bias["bias_va"] = np.zeros((1, 512), np.float16)
    bias["bias_vb"] = np.zeros((1, 512), np.float16)
    for lh in range(4):
        g = half * 4 + lh
        hs = slice(g * D, (g + 1) * D)
        wo_r_l.append(np.concatenate([WoT_r[hs, :], -WoT_i[hs, :]], axis=0))
        wo_i_l.append(np.concatenate([WoT_i[hs, :], WoT_r[hs, :]], axis=0))
        bias["bias_q"][lh] = np.concatenate([bq.real[hs], bq.imag[hs]])
        bias["bias_k1"][lh] = np.concatenate([bk.real[hs], -bk.imag[hs]])
        bias["bias_k2"][lh] = np.concatenate([bk.imag[hs], bk.real[hs]])
        bias["bias_va"][0, lh * 128:(lh + 1) * 128] = np.concatenate(
            [bv.real[hs], bv.imag[hs]])
        bias["bias_vb"][0, lh * 128:(lh + 1) * 128] = np.concatenate(
            [-bv.imag[hs], bv.real[hs]])

    inp = {
        "xt_r": xt_r, "xt_i": xt_i,
        "wq_a": wq_a, "wq_b": wq_b,
        "wk1_a": wk1_a, "wk1_b": wk1_b,
        "wk2_a": wk2_a, "wk2_b": wk2_b,
        "wv1_a": wv1_a, "wv1_b": wv1_b,
        "wv2_a": wv2_a, "wv2_b": wv2_b,
        "wo_r": np.stack(wo_r_l), "wo_i": np.stack(wo_i_l),
        "ones_in": np.ones((1, 512), np.float16),
    }
    inp.update(bias)
    return inp


def _enable_profiling():
    import sys
    import types
    if "antenv.axon_hooks" not in sys.modules:
        mod = types.ModuleType("antenv.axon_hooks")
        mod._hook = None
        mod.set_axon_ntff_profile_hook = lambda h: setattr(mod, "_hook", h)
        mod.get_axon_ntff_profile_hook = lambda: mod._hook
        sys.modules["antenv.axon_hooks"] = mod
        import antenv
        antenv.axon_hooks = mod
    from trn_agent_boot.trn_boot import _ntff_profile_via_ctypes
    sys.modules["antenv.axon_hooks"].set_axon_ntff_profile_hook(
        _ntff_profile_via_ctypes("/opt/axon/libaxon_pjrt.so"))
    import concourse.bass_utils as bu
    bu.upload_artifacts = lambda tmpdir: f"file://{tmpdir}"


def kernel(query, Wq, bq, Wk, bk, Wv, bv, Wo, bo):
    global last_exec_ns, last_scope_times
    from concourse.bass_utils import run_bass_kernel_spmd

    trace = os.environ.get("TRN_MHA_TRACE", "") == "1"
    if trace:
        _enable_profiling()

    if "nc" not in _prog_cache:
        _prog_cache["nc"] = _build_program()
    nc = _prog_cache["nc"]

    in_maps = []
    for c in range(N_CORES):
        b, half = c // 2, c % 2
        in_maps.append(_core_inputs(query, Wq, bq, Wk, bk, Wv, bv, Wo, bo,
                                    b, half))

    res = run_bass_kernel_spmd(nc, in_maps, list(range(N_CORES)), trace=trace)
    if trace:
        last_exec_ns = res.exec_time_ns
        last_scope_times = res.per_core_scope_times

    attn_output = np.zeros((B, T, C), np.complex64)
    attn_weights = np.zeros((B, H, T, T), np.complex64)
    for c in range(N_CORES):
        b, half = c // 2, c % 2
        r = res.results[c]
        attn_weights[b, half * 4:(half + 1) * 4] = r["attw_r"] + 1j * r["attw_i"]
        attn_output[b] += r["yp_r"] + 1j * r["yp_i"]
    attn_output += bo.astype(np.complex64)
    return (attn_output.astype(np.complex64),
            attn_weights.astype(np.complex64))


# revision 4
# speedup vs baseline: 1.0015x; 1.0015x over previous
"""Complex-valued multihead attention (B=4, T=1024, C=512, H=8) on 8 Trainium2
NeuronCores.

Sharding: core c = (b = c//2, half = c%2) handles batch b and heads
half*4 .. half*4+3 (tensor-parallel over heads within a batch).  The out_proj
is computed as per-core partials over each core's 256 head-dims and summed on
the host (the reduce of the head-TP all-gather), where the bo bias is added.

On-device math uses float32r (full-rate fp32 PE path, ~1e-4 component error).
Complex matmuls are computed as single K=128 matmuls by stacking real/imag
parts along the contraction axis, e.g.
  Sr = [Qr;Qi]^T-stationary x [Kr;-Ki]-moving  (t x s tile in one matmul).
Linear-layer biases are folded in as K=1 fp16 matmuls accumulating into PSUM.
"""
import os

import numpy as np

B, T, C, H = 4, 1024, 512, 8
D = C // H  # 64
O = 256     # head-dims per core (4 heads x 64)
SCALE = D ** (-0.5)
N_CORES = 8
IC = C // 128  # 4 contraction chunks over c_in
TP = T // 128  # 8 t-partition chunks
SF = T // 512  # 2 free-dim chunks of 512

_prog_cache = {}
last_exec_ns = None
last_scope_times = None


def _build_program():
    import concourse.bacc as bacc
    import concourse.tile as tile
    from concourse import mybir

    F32 = mybir.dt.float32
    F32R = mybir.dt.float32r
    F16 = mybir.dt.float16
    AF = mybir.ActivationFunctionType
    ALU = mybir.AluOpType

    nc = bacc.Bacc("TRN2", target_bir_lowering=False, debug=False,
                   num_devices=N_CORES)

    # ---- DRAM I/O ----
    xt_r = nc.dram_tensor("xt_r", [C, T], F32R, kind="ExternalInput").ap()
    xt_i = nc.dram_tensor("xt_i", [C, T], F32R, kind="ExternalInput").ap()
    wstk = {}
    for name in ("wq_a", "wq_b", "wk1_a", "wk1_b", "wk2_a", "wk2_b"):
        wstk[name] = nc.dram_tensor(name, [4, IC, 128, 128], F32R,
                                    kind="ExternalInput").ap()
    for name in ("wv1_a", "wv1_b", "wv2_a", "wv2_b"):
        wstk[name] = nc.dram_tensor(name, [IC, 128, 512], F32R,
                                    kind="ExternalInput").ap()
    wo_r = nc.dram_tensor("wo_r", [4, 128, C], F32R, kind="ExternalInput").ap()
    wo_i = nc.dram_tensor("wo_i", [4, 128, C], F32R, kind="ExternalInput").ap()
    bias_q = nc.dram_tensor("bias_q", [4, 128], F16, kind="ExternalInput").ap()
    bias_k1 = nc.dram_tensor("bias_k1", [4, 128], F16, kind="ExternalInput").ap()
    bias_k2 = nc.dram_tensor("bias_k2", [4, 128], F16, kind="ExternalInput").ap()
    bias_va = nc.dram_tensor("bias_va", [1, 512], F16, kind="ExternalInput").ap()
    bias_vb = nc.dram_tensor("bias_vb", [1, 512], F16, kind="ExternalInput").ap()
    ones_in = nc.dram_tensor("ones_in", [1, 512], F16, kind="ExternalInput").ap()

    attw_r = nc.dram_tensor("attw_r", [4, T, T], F32, kind="ExternalOutput").ap()
    attw_i = nc.dram_tensor("attw_i", [4, T, T], F32, kind="ExternalOutput").ap()
    yp_r = nc.dram_tensor("yp_r", [T, C], F32, kind="ExternalOutput").ap()
    yp_i = nc.dram_tensor("yp_i", [T, C], F32, kind="ExternalOutput").ap()

    with tile.TileContext(nc) as tc:
        with (
            tc.tile_pool(name="xt", bufs=1) as xt_pool,
            tc.tile_pool(name="wt", bufs=16) as wt_pool,
            tc.tile_pool(name="wv", bufs=8) as wv_pool,
            tc.tile_pool(name="wo", bufs=1) as wo_pool,
            tc.tile_pool(name="qk", bufs=2) as qk_pool,
            tc.tile_pool(name="vs", bufs=1) as vs_pool,
            tc.tile_pool(name="st", bufs=16) as st_pool,
            tc.tile_pool(name="sdr", bufs=4) as sdr_pool,
            tc.tile_pool(name="av", bufs=1) as av_pool,
            tc.tile_pool(name="yp", bufs=2) as yp_pool,
            tc.tile_pool(name="cst", bufs=1) as cst_pool,
            tc.tile_pool(name="pp", bufs=4, space="PSUM") as pp_pool,
            tc.tile_pool(name="pav", bufs=2, space="PSUM") as pav_pool,
        ):
            # ---- constants / inputs ----
            ones_sb = cst_pool.tile([1, 512], F16, tag="ones")
            nc.gpsimd.dma_start(ones_sb[:], ones_in[:])
            bias_sb = {}
            for nm, ap in (("bq", bias_q), ("bk1", bias_k1), ("bk2", bias_k2)):
                for lh in range(4):
                    t = cst_pool.tile([1, 128], F16, tag=f"{nm}{lh}")
                    nc.gpsimd.dma_start(t[:], ap[lh:lh + 1, :])
                    bias_sb[(nm, lh)] = t
            bva_sb = cst_pool.tile([1, 512], F16, tag="bva")
            nc.gpsimd.dma_start(bva_sb[:], bias_va[:])
            bvb_sb = cst_pool.tile([1, 512], F16, tag="bvb")
            nc.gpsimd.dma_start(bvb_sb[:], bias_vb[:])

            xtr = []
            xti = []
            for ic in range(IC):
                t_r = xt_pool.tile([128, T], F32R, tag=f"xtr{ic}")
                nc.gpsimd.dma_start(t_r[:], xt_r[ic * 128:(ic + 1) * 128, :])
                xtr.append(t_r)
                t_i = xt_pool.tile([128, T], F32R, tag=f"xti{ic}")
                nc.gpsimd.dma_start(t_i[:], xt_i[ic * 128:(ic + 1) * 128, :])
                xti.append(t_i)

            wo_sb = {}
            for lh in range(4):
                t = wo_pool.tile([128, C], F32R, tag=f"wor{lh}")
                nc.gpsimd.dma_start(t[:], wo_r[lh])
                wo_sb[("r", lh)] = t
                t = wo_pool.tile([128, C], F32R, tag=f"woi{lh}")
                nc.gpsimd.dma_start(t[:], wo_i[lh])
                wo_sb[("i", lh)] = t

            # ---- vstk projections, all 4 heads batched in the free dim ----
            # per tp: vstk_a = [Vr_h|Vi_h]x4 heads, vstk_b = [-Vi_h|Vr_h]x4
            vstk = {}
            for variant, (wa, wb, bseg) in {
                "a": ("wv1_a", "wv1_b", bva_sb),
                "b": ("wv2_a", "wv2_b", bvb_sb),
            }.items():
                wa_t = []
                wb_t = []
                for ic in range(IC):
                    t = wv_pool.tile([128, 512], F32R, tag="wv")
                    nc.gpsimd.dma_start(t[:], wstk[wa][ic])
                    wa_t.append(t)
                    t = wv_pool.tile([128, 512], F32R, tag="wv")
                    nc.gpsimd.dma_start(t[:], wstk[wb][ic])
                    wb_t.append(t)
                for tp in range(TP):
                    ps = pp_pool.tile([128, 512], F32, tag="pp")
                    for ic in range(IC):
                        nc.tensor.matmul(
                            ps[:], xtr[ic][:, tp * 128:(tp + 1) * 128],
                            wa_t[ic][:], start=(ic == 0), stop=False)
                        nc.tensor.matmul(
                            ps[:], xti[ic][:, tp * 128:(tp + 1) * 128],
                            wb_t[ic][:], start=False, stop=False)
                    nc.tensor.matmul(
                        ps[:], ones_sb[:, 0:128], bseg[:],
                        start=False, stop=True)
                    dst = vs_pool.tile([128, 512], F32R,
                                       tag=f"v{variant}_{tp}")
                    nc.scalar.activation(dst[:], ps[:], AF.Identity,
                                         bias=0.0, scale=1.0)
                    vstk[(variant, tp)] = dst

            avstk = {}
            # ---- per-head: Q/K projections, scores, AV ----
            for lh in range(4):
                qkt = {}
                for name, (wa, wb, bnm) in {
                    "qt": ("wq_a", "wq_b", "bq"),
                    "ktn": ("wk1_a", "wk1_b", "bk1"),
                    "kts": ("wk2_a", "wk2_b", "bk2"),
                }.items():
                    wa_t = []
                    wb_t = []
                    for ic in range(IC):
                        t = wt_pool.tile([128, 128], F32R, tag="w")
                        nc.gpsimd.dma_start(t[:], wstk[wa][lh, ic])
                        wa_t.append(t)
                        t = wt_pool.tile([128, 128], F32R, tag="w")
                        nc.gpsimd.dma_start(t[:], wstk[wb][lh, ic])
                        wb_t.append(t)
                    dst = qk_pool.tile([128, T], F32R, tag=name)
                    for tf in range(SF):
                        ps = pp_pool.tile([128, 512], F32, tag="pp")
                        for ic in range(IC):
                            nc.tensor.matmul(
                                ps[:], wa_t[ic][:],
                                xtr[ic][:, tf * 512:(tf + 1) * 512],
                                start=(ic == 0), stop=False)
                            nc.tensor.matmul(
                                ps[:], wb_t[ic][:],
                                xti[ic][:, tf * 512:(tf + 1) * 512],
                                start=False, stop=False)
                        nc.tensor.matmul(
                            ps[:], bias_sb[(bnm, lh)][:], ones_sb[:],
                            start=False, stop=True)
                        nc.scalar.activation(dst[:, tf * 512:(tf + 1) * 512],
                                             ps[:], AF.Identity,
                                             bias=0.0, scale=1.0)
                    qkt[name] = dst
                qt, ktn, kts = qkt["qt"], qkt["ktn"], qkt["kts"]

                # St' = relu(SCALE * S^T) tiles [s-part x t] (feeds AV), per
                # t-half to bound SBUF.
                for th in range(2):
                    stt = {}
                    for sp in range(8):
                        ps_r = pp_pool.tile([128, 512], F32, tag="pp")
                        nc.tensor.matmul(
                            ps_r[:], ktn[:, sp * 128:(sp + 1) * 128],
                            qt[:, th * 512:(th + 1) * 512],
                            start=True, stop=True)
                        ps_i = pp_pool.tile([128, 512], F32, tag="pp")
                        nc.tensor.matmul(
                            ps_i[:], kts[:, sp * 128:(sp + 1) * 128],
                            qt[:, th * 512:(th + 1) * 512],
                            start=True, stop=True)
                        st_r = st_pool.tile([128, 512], F32R, tag="st")
                        nc.scalar.activation(st_r[:], ps_r[:], AF.Relu,
                                             bias=0.0, scale=SCALE)
                        st_i = st_pool.tile([128, 512], F32R, tag="st")
                        nc.scalar.activation(st_i[:], ps_i[:], AF.Relu,
                                             bias=0.0, scale=SCALE)
                        stt[("r", sp)] = st_r
                        stt[("i", sp)] = st_i

                    # S' = relu(SCALE * S) tiles [t-part x s] for the t-half;
                    # streamed straight to the attn_weights output.
                    for tpl in range(4):
                        tp = th * 4 + tpl
                        for sf in range(SF):
                            ps_r = pp_pool.tile([128, 512], F32, tag="pp")
                            nc.tensor.matmul(
                                ps_r[:], qt[:, tp * 128:(tp + 1) * 128],
                                ktn[:, sf * 512:(sf + 1) * 512],
                                start=True, stop=True)
                            ps_i = pp_pool.tile([128, 512], F32, tag="pp")
                            nc.tensor.matmul(
                                ps_i[:], qt[:, tp * 128:(tp + 1) * 128],
                                kts[:, sf * 512:(sf + 1) * 512],
                                start=True, stop=True)
                            s_r = sdr_pool.tile([128, 512], F32, tag="sdr")
                            nc.vector.tensor_scalar(s_r[:], ps_r[:], SCALE, 0.0,
                                                    ALU.mult, ALU.max)
                            s_i = sdr_pool.tile([128, 512], F32, tag="sdr")
                            nc.vector.tensor_scalar(s_i[:], ps_i[:], SCALE, 0.0,
                                                    ALU.mult, ALU.max)
                            nc.sync.dma_start(
                                attw_r[lh, tp * 128:(tp + 1) * 128,
                                       sf * 512:(sf + 1) * 512], s_r[:])
                            nc.gpsimd.dma_start(
                                attw_i[lh, tp * 128:(tp + 1) * 128,
                                       sf * 512:(sf + 1) * 512], s_i[:])

                    # AV for this t-half: avstk psum rows = [AVt_r; AVt_i]
                    ps_av = pav_pool.tile([128, 512], F32, tag="pav")
                    for sp in range(8):
                        nc.tensor.matmul(
                            ps_av[:],
                            vstk[("a", sp)][:, lh * 128:(lh + 1) * 128],
                            stt[("r", sp)][:],
                            start=(sp == 0), stop=False)
                        nc.tensor.matmul(
                            ps_av[:],
                            vstk[("b", sp)][:, lh * 128:(lh + 1) * 128],
                            stt[("i", sp)][:],
                            start=False, stop=(sp == 7))
                    if th == 0:
                        av_sb = av_pool.tile([128, T], F32R, tag=f"av{lh}")
                        avstk[lh] = av_sb
                    nc.scalar.activation(
                        avstk[lh][:, th * 512:(th + 1) * 512], ps_av[:],
                        AF.Identity, bias=0.0, scale=1.0)

            # ---- out_proj partials ----
            for tp in range(TP):
                ps_r = pp_pool.tile([128, 512], F32, tag="pp")
                ps_i = pp_pool.tile([128, 512], F32, tag="pp")
                for lh in range(4):
                    nc.tensor.matmul(ps_r[:],
                                     avstk[lh][:, tp * 128:(tp + 1) * 128],
                                     wo_sb[("r", lh)][:],
                                     start=(lh == 0), stop=(lh == 3))
                for lh in range(4):
                    nc.tensor.matmul(ps_i[:],
                                     avstk[lh][:, tp * 128:(tp + 1) * 128],
                                     wo_sb[("i", lh)][:],
                                     start=(lh == 0), stop=(lh == 3))
                o_r = yp_pool.tile([128, 512], F32, tag="yp")
                nc.vector.tensor_copy(o_r[:], ps_r[:])
                nc.sync.dma_start(yp_r[tp * 128:(tp + 1) * 128, :], o_r[:])
                o_i = yp_pool.tile([128, 512], F32, tag="yp")
                nc.vector.tensor_copy(o_i[:], ps_i[:])
                nc.gpsimd.dma_start(yp_i[tp * 128:(tp + 1) * 128, :], o_i[:])

    nc.compile()
    return nc


def _head_stacks(Wr, Wi, hs):
    # -> per-head (512, 128) transposed stationary blocks
    return Wr[hs, :].T.copy(), Wi[hs, :].T.copy()


def _as_chunks(m):
    # (512, 128) -> (IC, 128, 128)
    return np.ascontiguousarray(m.reshape(IC, 128, 128))


def _core_inputs(query, Wq, bq, Wk, bk, Wv, bv, Wo, bo, b, half):
    f32 = np.float32
    x = query[b]
    xt_r = np.ascontiguousarray(x.real.T).astype(f32)
    xt_i = np.ascontiguousarray(x.imag.T).astype(f32)

    WqT_r, WqT_i = Wq.real.T.astype(f32), Wq.imag.T.astype(f32)
    WkT_r, WkT_i = Wk.real.T.astype(f32), Wk.imag.T.astype(f32)
    WvT_r, WvT_i = Wv.real.T.astype(f32), Wv.imag.T.astype(f32)
    WoT_r, WoT_i = Wo.real.T.astype(f32), Wo.imag.T.astype(f32)

    def stacks(WT_r, WT_i, a_pair, b_pair):
        # a_pair/b_pair: ((sign, 'r'|'i'), (sign, 'i'|'r')) selecting the
        # [left|right] 64-col halves of the xr-term and xi-term stationaries
        src = {"r": WT_r, "i": WT_i}
        a_l = []
        b_l = []
        for lh in range(4):
            g = half * 4 + lh
            hs = slice(g * D, (g + 1) * D)
            cols_a = np.concatenate(
                [s * src[k][:, hs] for s, k in a_pair], axis=1)
            cols_b = np.concatenate(
                [s * src[k][:, hs] for s, k in b_pair], axis=1)
            a_l.append(_as_chunks(cols_a))
            b_l.append(_as_chunks(cols_b))
        return np.stack(a_l), np.stack(b_l)

    wq_a, wq_b = stacks(WqT_r, WqT_i, ((1, "r"), (1, "i")), ((-1, "i"), (1, "r")))
    wk1_a, wk1_b = stacks(WkT_r, WkT_i, ((1, "r"), (-1, "i")), ((-1, "i"), (-1, "r")))
    wk2_a, wk2_b = stacks(WkT_r, WkT_i, ((1, "i"), (1, "r")), ((1, "r"), (-1, "i")))

    def vstacks(a_pair, b_pair):
        src_ = {"r": WvT_r, "i": WvT_i}
        cols_a = []
        cols_b = []
        for lh in range(4):
            g = half * 4 + lh
            hs = slice(g * D, (g + 1) * D)
            cols_a.append(np.concatenate(
                [s * src_[k][:, hs] for s, k in a_pair], axis=1))
            cols_b.append(np.concatenate(
                [s * src_[k][:, hs] for s, k in b_pair], axis=1))
        a = np.ascontiguousarray(
            np.concatenate(cols_a, axis=1).reshape(IC, 128, 512))
        bm = np.ascontiguousarray(
            np.concatenate(cols_b, axis=1).reshape(IC, 128, 512))
        return a, bm

    wv1_a, wv1_b = vstacks(((1, "r"), (1, "i")), ((-1, "i"), (1, "r")))
    wv2_a, wv2_b = vstacks(((-1, "i"), (1, "r")), ((-1, "r"), (-1, "i")))

    wo_r_l = []
    wo_i_l = []
    bias = {}
    for nm in ("bias_q", "bias_k1", "bias_k2"):
        bias[nm] = np.zeros((4, 128), np.float16)
    bias["bias_va"] = np.zeros((1, 512), np.float16)
    bias["bias_vb"] = np.zeros((1, 512), np.float16)
    for lh in range(4):
        g = half * 4 + lh
        hs = slice(g * D, (g + 1) * D)
        wo_r_l.append(np.concatenate([WoT_r[hs, :], -WoT_i[hs, :]], axis=0))
        wo_i_l.append(np.concatenate([WoT_i[hs, :], WoT_r[hs, :]], axis=0))
        bias["bias_q"][lh] = np.concatenate([bq.real[hs], bq.imag[hs]])
        bias["bias_k1"][lh] = np.concatenate([bk.real[hs], -bk.imag[hs]])
        bias["bias_k2"][lh] = np.concatenate([bk.imag[hs], bk.real[hs]])
        bias["bias_va"][0, lh * 128:(lh + 1) * 128] = np.concatenate(
            [bv.real[hs], bv.imag[hs]])
        bias["bias_vb"][0, lh * 128:(lh + 1) * 128] = np.concatenate(
            [-bv.imag[hs], bv.real[hs]])

    inp = {
        "xt_r": xt_r, "xt_i": xt_i,
        "wq_a": wq_a, "wq_b": wq_b,
        "wk1_a": wk1_a, "wk1_b": wk1_b,
        "wk2_a": wk2_a, "wk2_b": wk2_b,
        "wv1_a": wv1_a, "wv1_b": wv1_b,
        "wv2_a": wv2_a, "wv2_b": wv2_b,
        "wo_r": np.stack(wo_r_l), "wo_i": np.stack(wo_i_l),
        "ones_in": np.ones((1, 512), np.float16),
    }
    inp.update(bias)
    return inp


def _enable_profiling():
    import sys
    import types
    if "antenv.axon_hooks" not in sys.modules:
        mod = types.ModuleType("antenv.axon_hooks")
        mod._hook = None
        mod.set_axon_ntff_profile_hook = lambda h: setattr(mod, "_hook", h)
        mod.get_axon_ntff_profile_hook = lambda: mod._hook
        sys.modules["antenv.axon_hooks"] = mod
        import antenv
        antenv.axon_hooks = mod
    from trn_agent_boot.trn_boot import _ntff_profile_via_ctypes
    sys.modules["antenv.axon_hooks"].set_axon_ntff_profile_hook(
        _ntff_profile_via_ctypes("/opt/axon/libaxon_pjrt.so"))
    import concourse.bass_utils as bu
    bu.upload_artifacts = lambda tmpdir: f"file://{tmpdir}"


def kernel(query, Wq, bq, Wk, bk, Wv, bv, Wo, bo):
    global last_exec_ns, last_scope_times
    from concourse.bass_utils import run_bass_kernel_spmd

    trace = os.environ.get("TRN_MHA_TRACE", "") == "1"
    if trace:
        _enable_profiling()

    if "nc" not in _prog_cache:
        _prog_cache["nc"] = _build_program()
    nc = _prog_cache["nc"]

    in_maps = []
    for c in range(N_CORES):
        b, half = c // 2, c % 2
        in_maps.append(_core_inputs(query, Wq, bq, Wk, bk, Wv, bv, Wo, bo,
                                    b, half))

    res = run_bass_kernel_spmd(nc, in_maps, list(range(N_CORES)), trace=trace)
    _prog_cache["last_res"] = res
    if trace:
        last_exec_ns = res.exec_time_ns
        last_scope_times = res.per_core_scope_times

    attn_output = np.zeros((B, T, C), np.complex64)
    attn_weights = np.zeros((B, H, T, T), np.complex64)
    for c in range(N_CORES):
        b, half = c // 2, c % 2
        r = res.results[c]
        attn_weights[b, half * 4:(half + 1) * 4] = r["attw_r"] + 1j * r["attw_i"]
        attn_output[b] += r["yp_r"] + 1j * r["yp_i"]
    attn_output += bo.astype(np.complex64)
    return (attn_output.astype(np.complex64),
            attn_weights.astype(np.complex64))


# revision 5
# speedup vs baseline: 1.2842x; 1.2823x over previous
"""Complex-valued multihead attention (B=4, T=1024, C=512, H=8) on 8 Trainium2
NeuronCores.

Sharding: core c = (b = c//2, half = c%2) handles batch b and heads
half*4 .. half*4+3 (tensor-parallel over heads within a batch).  The out_proj
is computed as per-core partials over each core's 256 head-dims and summed on
the host (the reduce of the head-TP all-gather), where the bo bias is added.

On-device math uses float32r (full-rate fp32 PE path, ~1e-4 component error).
Complex matmuls are computed as single K=128 matmuls by stacking real/imag
parts along the contraction axis, e.g.
  Sr = [Qr;Qi]^T-stationary x [Kr;-Ki]-moving  (t x s tile in one matmul).
Linear-layer biases are folded in as K=1 fp16 matmuls accumulating into PSUM.
"""
import os

import numpy as np

B, T, C, H = 4, 1024, 512, 8
D = C // H  # 64
O = 256     # head-dims per core (4 heads x 64)
SCALE = D ** (-0.5)
N_CORES = 8
IC = C // 128  # 4 contraction chunks over c_in
TP = T // 128  # 8 t-partition chunks
SF = T // 512  # 2 free-dim chunks of 512

_prog_cache = {}
last_exec_ns = None
last_scope_times = None


def _build_program():
    import concourse.bacc as bacc
    import concourse.tile as tile
    from concourse import mybir

    F32 = mybir.dt.float32
    F32R = mybir.dt.float32r
    F16 = mybir.dt.float16
    AF = mybir.ActivationFunctionType
    ALU = mybir.AluOpType

    nc = bacc.Bacc("TRN2", target_bir_lowering=False, debug=False,
                   num_devices=N_CORES)

    # ---- DRAM I/O ----
    xt_r = nc.dram_tensor("xt_r", [C, T], F32R, kind="ExternalInput").ap()
    xt_i = nc.dram_tensor("xt_i", [C, T], F32R, kind="ExternalInput").ap()
    wstk = {}
    for name in ("wq_a", "wq_b", "wk1_a", "wk1_b", "wk2_a", "wk2_b"):
        wstk[name] = nc.dram_tensor(name, [4, IC, 128, 128], F32R,
                                    kind="ExternalInput").ap()
    for name in ("wv1_a", "wv1_b", "wv2_a", "wv2_b"):
        wstk[name] = nc.dram_tensor(name, [IC, 128, 512], F32R,
                                    kind="ExternalInput").ap()
    wo_r = nc.dram_tensor("wo_r", [4, 128, C], F32R, kind="ExternalInput").ap()
    wo_i = nc.dram_tensor("wo_i", [4, 128, C], F32R, kind="ExternalInput").ap()
    bias_q = nc.dram_tensor("bias_q", [4, 128], F16, kind="ExternalInput").ap()
    bias_k1 = nc.dram_tensor("bias_k1", [4, 128], F16, kind="ExternalInput").ap()
    bias_k2 = nc.dram_tensor("bias_k2", [4, 128], F16, kind="ExternalInput").ap()
    bias_va = nc.dram_tensor("bias_va", [1, 512], F16, kind="ExternalInput").ap()
    bias_vb = nc.dram_tensor("bias_vb", [1, 512], F16, kind="ExternalInput").ap()
    ones_in = nc.dram_tensor("ones_in", [1, 512], F16, kind="ExternalInput").ap()

    attw_r = nc.dram_tensor("attw_r", [4, T, T], F32, kind="ExternalOutput").ap()
    attw_i = nc.dram_tensor("attw_i", [4, T, T], F32, kind="ExternalOutput").ap()
    yp_r = nc.dram_tensor("yp_r", [T, C], F32, kind="ExternalOutput").ap()
    yp_i = nc.dram_tensor("yp_i", [T, C], F32, kind="ExternalOutput").ap()

    with tile.TileContext(nc) as tc:
        with (
            tc.tile_pool(name="xt", bufs=1) as xt_pool,
            tc.tile_pool(name="wt", bufs=16) as wt_pool,
            tc.tile_pool(name="wv", bufs=8) as wv_pool,
            tc.tile_pool(name="wo", bufs=1) as wo_pool,
            tc.tile_pool(name="qk", bufs=2) as qk_pool,
            tc.tile_pool(name="vs", bufs=1) as vs_pool,
            tc.tile_pool(name="st", bufs=16) as st_pool,
            tc.tile_pool(name="sdr", bufs=4) as sdr_pool,
            tc.tile_pool(name="av", bufs=1) as av_pool,
            tc.tile_pool(name="yp", bufs=2) as yp_pool,
            tc.tile_pool(name="cst", bufs=1) as cst_pool,
            tc.tile_pool(name="pp", bufs=6, space="PSUM") as pp_pool,
            tc.tile_pool(name="pav", bufs=2, space="PSUM") as pav_pool,
        ):
            # ---- constants / inputs ----
            ones_sb = cst_pool.tile([1, 512], F16, tag="ones")
            nc.gpsimd.dma_start(ones_sb[:], ones_in[:])
            bias_sb = {}
            for nm, ap in (("bq", bias_q), ("bk1", bias_k1), ("bk2", bias_k2)):
                for lh in range(4):
                    t = cst_pool.tile([1, 128], F16, tag=f"{nm}{lh}")
                    nc.gpsimd.dma_start(t[:], ap[lh:lh + 1, :])
                    bias_sb[(nm, lh)] = t
            bva_sb = cst_pool.tile([1, 512], F16, tag="bva")
            nc.gpsimd.dma_start(bva_sb[:], bias_va[:])
            bvb_sb = cst_pool.tile([1, 512], F16, tag="bvb")
            nc.gpsimd.dma_start(bvb_sb[:], bias_vb[:])

            xtr = []
            xti = []
            for ic in range(IC):
                t_r = xt_pool.tile([128, T], F32R, tag=f"xtr{ic}")
                nc.gpsimd.dma_start(t_r[:], xt_r[ic * 128:(ic + 1) * 128, :])
                xtr.append(t_r)
                t_i = xt_pool.tile([128, T], F32R, tag=f"xti{ic}")
                nc.gpsimd.dma_start(t_i[:], xt_i[ic * 128:(ic + 1) * 128, :])
                xti.append(t_i)

            wo_sb = {}
            for lh in range(4):
                t = wo_pool.tile([128, C], F32R, tag=f"wor{lh}")
                nc.gpsimd.dma_start(t[:], wo_r[lh])
                wo_sb[("r", lh)] = t
                t = wo_pool.tile([128, C], F32R, tag=f"woi{lh}")
                nc.gpsimd.dma_start(t[:], wo_i[lh])
                wo_sb[("i", lh)] = t

            # ---- vstk projections, all 4 heads batched in the free dim ----
            # per tp: vstk_a = [Vr_h|Vi_h]x4 heads, vstk_b = [-Vi_h|Vr_h]x4
            vstk = {}
            scope_vproj = nc.named_scope("vproj")
            scope_vproj.__enter__()
            for variant, (wa, wb, bseg) in {
                "a": ("wv1_a", "wv1_b", bva_sb),
                "b": ("wv2_a", "wv2_b", bvb_sb),
            }.items():
                wa_t = []
                wb_t = []
                for ic in range(IC):
                    t = wv_pool.tile([128, 512], F32R, tag="wv")
                    nc.gpsimd.dma_start(t[:], wstk[wa][ic])
                    wa_t.append(t)
                    t = wv_pool.tile([128, 512], F32R, tag="wv")
                    nc.gpsimd.dma_start(t[:], wstk[wb][ic])
                    wb_t.append(t)
                for tp in range(TP):
                    ps = pp_pool.tile([128, 512], F32, tag="pp")
                    for ic in range(IC):
                        nc.tensor.matmul(
                            ps[:], xtr[ic][:, tp * 128:(tp + 1) * 128],
                            wa_t[ic][:], start=(ic == 0), stop=False)
                        nc.tensor.matmul(
                            ps[:], xti[ic][:, tp * 128:(tp + 1) * 128],
                            wb_t[ic][:], start=False, stop=False)
                    nc.tensor.matmul(
                        ps[:], ones_sb[:, 0:128], bseg[:],
                        start=False, stop=True)
                    dst = vs_pool.tile([128, 512], F32R,
                                       tag=f"v{variant}_{tp}")
                    nc.scalar.activation(dst[:], ps[:], AF.Identity,
                                         bias=0.0, scale=1.0)
                    vstk[(variant, tp)] = dst

            scope_vproj.__exit__(None, None, None)
            avstk = {}
            # ---- per-head: Q/K projections, scores, AV ----
            for lh in range(4):
                scope_h = nc.named_scope(f"head{lh}")
                scope_h.__enter__()
                qkt = {}
                for name, (wa, wb, bnm) in {
                    "qt": ("wq_a", "wq_b", "bq"),
                    "ktn": ("wk1_a", "wk1_b", "bk1"),
                    "kts": ("wk2_a", "wk2_b", "bk2"),
                }.items():
                    wa_t = []
                    wb_t = []
                    for ic in range(IC):
                        t = wt_pool.tile([128, 128], F32R, tag="w")
                        nc.gpsimd.dma_start(t[:], wstk[wa][lh, ic])
                        wa_t.append(t)
                        t = wt_pool.tile([128, 128], F32R, tag="w")
                        nc.gpsimd.dma_start(t[:], wstk[wb][lh, ic])
                        wb_t.append(t)
                    dst = qk_pool.tile([128, T], F32R, tag=name)
                    for tf in range(SF):
                        ps = pp_pool.tile([128, 512], F32, tag="pp")
                        for ic in range(IC):
                            nc.tensor.matmul(
                                ps[:], wa_t[ic][:],
                                xtr[ic][:, tf * 512:(tf + 1) * 512],
                                start=(ic == 0), stop=False)
                            nc.tensor.matmul(
                                ps[:], wb_t[ic][:],
                                xti[ic][:, tf * 512:(tf + 1) * 512],
                                start=False, stop=False)
                        nc.tensor.matmul(
                            ps[:], bias_sb[(bnm, lh)][:], ones_sb[:],
                            start=False, stop=True)
                        nc.scalar.activation(dst[:, tf * 512:(tf + 1) * 512],
                                             ps[:], AF.Identity,
                                             bias=0.0, scale=1.0)
                    qkt[name] = dst
                qt, ktn, kts = qkt["qt"], qkt["ktn"], qkt["kts"]

                # Scores for this t-half: St' tiles [s-part x t] (ACT
                # drains, feed AV) interleaved with S' tiles [t-part x s]
                # (DVE drains, stream to attn_weights) so both drain
                # engines run concurrently.
                for th in range(2):
                    stt = {}
                    for k in range(8):
                        sp = k
                        ps_r = pp_pool.tile([128, 512], F32, tag="pp")
                        nc.tensor.matmul(
                            ps_r[:], ktn[:, sp * 128:(sp + 1) * 128],
                            qt[:, th * 512:(th + 1) * 512],
                            start=True, stop=True)
                        ps_i = pp_pool.tile([128, 512], F32, tag="pp")
                        nc.tensor.matmul(
                            ps_i[:], kts[:, sp * 128:(sp + 1) * 128],
                            qt[:, th * 512:(th + 1) * 512],
                            start=True, stop=True)
                        st_r = st_pool.tile([128, 512], F32R, tag="st")
                        nc.scalar.activation(st_r[:], ps_r[:], AF.Relu,
                                             bias=0.0, scale=SCALE)
                        st_i = st_pool.tile([128, 512], F32R, tag="st")
                        nc.scalar.activation(st_i[:], ps_i[:], AF.Relu,
                                             bias=0.0, scale=SCALE)
                        stt[("r", sp)] = st_r
                        stt[("i", sp)] = st_i

                        tp = th * 4 + k // 2
                        sf = k % 2
                        ps_sr = pp_pool.tile([128, 512], F32, tag="pp")
                        nc.tensor.matmul(
                            ps_sr[:], qt[:, tp * 128:(tp + 1) * 128],
                            ktn[:, sf * 512:(sf + 1) * 512],
                            start=True, stop=True)
                        ps_si = pp_pool.tile([128, 512], F32, tag="pp")
                        nc.tensor.matmul(
                            ps_si[:], qt[:, tp * 128:(tp + 1) * 128],
                            kts[:, sf * 512:(sf + 1) * 512],
                            start=True, stop=True)
                        s_r = sdr_pool.tile([128, 512], F32, tag="sdr")
                        nc.vector.tensor_scalar(s_r[:], ps_sr[:], SCALE, 0.0,
                                                ALU.mult, ALU.max)
                        s_i = sdr_pool.tile([128, 512], F32, tag="sdr")
                        nc.vector.tensor_scalar(s_i[:], ps_si[:], SCALE, 0.0,
                                                ALU.mult, ALU.max)
                        nc.sync.dma_start(
                            attw_r[lh, tp * 128:(tp + 1) * 128,
                                   sf * 512:(sf + 1) * 512], s_r[:])
                        nc.sync.dma_start(
                            attw_i[lh, tp * 128:(tp + 1) * 128,
                                   sf * 512:(sf + 1) * 512], s_i[:])

                    # AV for this t-half: avstk psum rows = [AVt_r; AVt_i]
                    ps_av = pav_pool.tile([128, 512], F32, tag="pav")
                    for sp in range(8):
                        nc.tensor.matmul(
                            ps_av[:],
                            vstk[("a", sp)][:, lh * 128:(lh + 1) * 128],
                            stt[("r", sp)][:],
                            start=(sp == 0), stop=False)
                        nc.tensor.matmul(
                            ps_av[:],
                            vstk[("b", sp)][:, lh * 128:(lh + 1) * 128],
                            stt[("i", sp)][:],
                            start=False, stop=(sp == 7))
                    if th == 0:
                        av_sb = av_pool.tile([128, T], F32R, tag=f"av{lh}")
                        avstk[lh] = av_sb
                    nc.scalar.activation(
                        avstk[lh][:, th * 512:(th + 1) * 512], ps_av[:],
                        AF.Identity, bias=0.0, scale=1.0)

                scope_h.__exit__(None, None, None)
            # ---- out_proj partials ----
            scope_yp = nc.named_scope("yp")
            scope_yp.__enter__()
            for tp in range(TP):
                ps_r = pp_pool.tile([128, 512], F32, tag="pp")
                ps_i = pp_pool.tile([128, 512], F32, tag="pp")
                for lh in range(4):
                    nc.tensor.matmul(ps_r[:],
                                     avstk[lh][:, tp * 128:(tp + 1) * 128],
                                     wo_sb[("r", lh)][:],
                                     start=(lh == 0), stop=(lh == 3))
                for lh in range(4):
                    nc.tensor.matmul(ps_i[:],
                                     avstk[lh][:, tp * 128:(tp + 1) * 128],
                                     wo_sb[("i", lh)][:],
                                     start=(lh == 0), stop=(lh == 3))
                o_r = yp_pool.tile([128, 512], F32, tag="yp")
                nc.vector.tensor_copy(o_r[:], ps_r[:])
                nc.sync.dma_start(yp_r[tp * 128:(tp + 1) * 128, :], o_r[:])
                o_i = yp_pool.tile([128, 512], F32, tag="yp")
                nc.vector.tensor_copy(o_i[:], ps_i[:])
                nc.gpsimd.dma_start(yp_i[tp * 128:(tp + 1) * 128, :], o_i[:])

            scope_yp.__exit__(None, None, None)
    nc.compile()
    return nc


def _head_stacks(Wr, Wi, hs):
    # -> per-head (512, 128) transposed stationary blocks
    return Wr[hs, :].T.copy(), Wi[hs, :].T.copy()


def _as_chunks(m):
    # (512, 128) -> (IC, 128, 128)
    return np.ascontiguousarray(m.reshape(IC, 128, 128))


def _core_inputs(query, Wq, bq, Wk, bk, Wv, bv, Wo, bo, b, half):
    f32 = np.float32
    x = query[b]
    xt_r = np.ascontiguousarray(x.real.T).astype(f32)
    xt_i = np.ascontiguousarray(x.imag.T).astype(f32)

    WqT_r, WqT_i = Wq.real.T.astype(f32), Wq.imag.T.astype(f32)
    WkT_r, WkT_i = Wk.real.T.astype(f32), Wk.imag.T.astype(f32)
    WvT_r, WvT_i = Wv.real.T.astype(f32), Wv.imag.T.astype(f32)
    WoT_r, WoT_i = Wo.real.T.astype(f32), Wo.imag.T.astype(f32)

    def stacks(WT_r, WT_i, a_pair, b_pair):
        # a_pair/b_pair: ((sign, 'r'|'i'), (sign, 'i'|'r')) selecting the
        # [left|right] 64-col halves of the xr-term and xi-term stationaries
        src = {"r": WT_r, "i": WT_i}
        a_l = []
        b_l = []
        for lh in range(4):
            g = half * 4 + lh
            hs = slice(g * D, (g + 1) * D)
            cols_a = np.concatenate(
                [s * src[k][:, hs] for s, k in a_pair], axis=1)
            cols_b = np.concatenate(
                [s * src[k][:, hs] for s, k in b_pair], axis=1)
            a_l.append(_as_chunks(cols_a))
            b_l.append(_as_chunks(cols_b))
        return np.stack(a_l), np.stack(b_l)

    wq_a, wq_b = stacks(WqT_r, WqT_i, ((1, "r"), (1, "i")), ((-1, "i"), (1, "r")))
    wk1_a, wk1_b = stacks(WkT_r, WkT_i, ((1, "r"), (-1, "i")), ((-1, "i"), (-1, "r")))
    wk2_a, wk2_b = stacks(WkT_r, WkT_i, ((1, "i"), (1, "r")), ((1, "r"), (-1, "i")))

    def vstacks(a_pair, b_pair):
        src_ = {"r": WvT_r, "i": WvT_i}
        cols_a = []
        cols_b = []
        for lh in range(4):
            g = half * 4 + lh
            hs = slice(g * D, (g + 1) * D)
            cols_a.append(np.concatenate(
                [s * src_[k][:, hs] for s, k in a_pair], axis=1))
            cols_b.append(np.concatenate(
                [s * src_[k][:, hs] for s, k in b_pair], axis=1))
        a = np.ascontiguousarray(
            np.concatenate(cols_a, axis=1).reshape(IC, 128, 512))
        bm = np.ascontiguousarray(
            np.concatenate(cols_b, axis=1).reshape(IC, 128, 512))
        return a, bm

    wv1_a, wv1_b = vstacks(((1, "r"), (1, "i")), ((-1, "i"), (1, "r")))
    wv2_a, wv2_b = vstacks(((-1, "i"), (1, "r")), ((-1, "r"), (-1, "i")))

    wo_r_l = []
    wo_i_l = []
    bias = {}
    for nm in ("bias_q", "bias_k1", "bias_k2"):
        bias[nm] = np.zeros((4, 128), np.float16)
    bias["bias_va"] = np.zeros((1, 512), np.float16)
    bias["bias_vb"] = np.zeros((1, 512), np.float16)
    for lh in range(4):
        g = half * 4 + lh
        hs = slice(g * D, (g + 1) * D)
        wo_r_l.append(np.concatenate([WoT_r[hs, :], -WoT_i[hs, :]], axis=0))
        wo_i_l.append(np.concatenate([WoT_i[hs, :], WoT_r[hs, :]], axis=0))
        bias["bias_q"][lh] = np.concatenate([bq.real[hs], bq.imag[hs]])
        bias["bias_k1"][lh] = np.concatenate([bk.real[hs], -bk.imag[hs]])
        bias["bias_k2"][lh] = np.concatenate([bk.imag[hs], bk.real[hs]])
        bias["bias_va"][0, lh * 128:(lh + 1) * 128] = np.concatenate(
            [bv.real[hs], bv.imag[hs]])
        bias["bias_vb"][0, lh * 128:(lh + 1) * 128] = np.concatenate(
            [-bv.imag[hs], bv.real[hs]])

    inp = {
        "xt_r": xt_r, "xt_i": xt_i,
        "wq_a": wq_a, "wq_b": wq_b,
        "wk1_a": wk1_a, "wk1_b": wk1_b,
        "wk2_a": wk2_a, "wk2_b": wk2_b,
        "wv1_a": wv1_a, "wv1_b": wv1_b,
        "wv2_a": wv2_a, "wv2_b": wv2_b,
        "wo_r": np.stack(wo_r_l), "wo_i": np.stack(wo_i_l),
        "ones_in": np.ones((1, 512), np.float16),
    }
    inp.update(bias)
    return inp


def _enable_profiling():
    import sys
    import types
    if "antenv.axon_hooks" not in sys.modules:
        mod = types.ModuleType("antenv.axon_hooks")
        mod._hook = None
        mod.set_axon_ntff_profile_hook = lambda h: setattr(mod, "_hook", h)
        mod.get_axon_ntff_profile_hook = lambda: mod._hook
        sys.modules["antenv.axon_hooks"] = mod
        import antenv
        antenv.axon_hooks = mod
    from trn_agent_boot.trn_boot import _ntff_profile_via_ctypes
    sys.modules["antenv.axon_hooks"].set_axon_ntff_profile_hook(
        _ntff_profile_via_ctypes("/opt/axon/libaxon_pjrt.so"))
    import concourse.bass_utils as bu
    bu.upload_artifacts = lambda tmpdir: f"file://{tmpdir}"


def kernel(query, Wq, bq, Wk, bk, Wv, bv, Wo, bo):
    global last_exec_ns, last_scope_times
    from concourse.bass_utils import run_bass_kernel_spmd

    trace = os.environ.get("TRN_MHA_TRACE", "") == "1"
    if trace:
        _enable_profiling()

    if "nc" not in _prog_cache:
        _prog_cache["nc"] = _build_program()
    nc = _prog_cache["nc"]

    in_maps = []
    for c in range(N_CORES):
        b, half = c // 2, c % 2
        in_maps.append(_core_inputs(query, Wq, bq, Wk, bk, Wv, bv, Wo, bo,
                                    b, half))

    res = run_bass_kernel_spmd(nc, in_maps, list(range(N_CORES)), trace=trace)
    _prog_cache["last_res"] = res
    if trace:
        last_exec_ns = res.exec_time_ns
        last_scope_times = res.per_core_scope_times

    attn_output = np.zeros((B, T, C), np.complex64)
    attn_weights = np.zeros((B, H, T, T), np.complex64)
    for c in range(N_CORES):
        b, half = c // 2, c % 2
        r = res.results[c]
        attn_weights[b, half * 4:(half + 1) * 4] = r["attw_r"] + 1j * r["attw_i"]
        attn_output[b] += r["yp_r"] + 1j * r["yp_i"]
    attn_output += bo.astype(np.complex64)
    return (attn_output.astype(np.complex64),
            attn_weights.astype(np.complex64))
